# revision 1
# baseline (speedup 1.0000x reference)
"""Two-layer GAT on 8 Trainium2 cores via Bass/Tile.

Strategy (dst-node graph partition, per the sharding hint):
- Nodes are split into 8 contiguous ranges (6250 per core); every edge is
  owned by the core that owns its dst node.
- Launch A (layer 1): each core redundantly computes the dense part
  hx = x @ [W1 | W1@blockdiag(a_src1) | W1@blockdiag(a_dst1)] into fp16
  DRAM tables, then aggregates its own edges with one-hot PE matmuls:
    * node rows are stored in two tables split at LOSPLIT so dma_gather's
      int16 indices stay < 32768; row width 384 fp16 (768B, 256B-aligned).
    * edges are bucketed per (core, 128-dst-node group) and split into
      lo/hi sub-buckets by src row; each padded to EB_LO/EB_HI blocks of
      128 edges (dummy edges point at sentinel rows with a_src = -30000 so
      exp() == 0).
    * per 128-edge block: M[e,j] = (dst_local[e] == j) via DVE is_equal
      against an iota constant; M_T via PE transpose; per-edge a_dst from
      M_T.T @ ad_group (a [128,8] dense gather per group); logits, exp
      (shifted by -6), and one PE matmul U += M.T @ [ex*h | ex] accumulated
      in PSUM over the group's blocks.
    * group epilogue: h1 = elu(U[:,:256]/(U[:,256:264]+1e-16) + b1),
      transposed on PE, written as hl1T [256, 6250] fp16.
- Host: concatenates the 8 hl1T shards (pure data movement).
- Launch B (layer 2): same structure, OUT=32, one head, fp32 edge math,
  hx2 = hl1 @ [W2 | W2@a_src2.T | W2@a_dst2.T] from hl1T tiles (no
  transposes needed).
"""

import sys
for _p in ("/opt/trn_rl_repo",):
    if _p not in sys.path:
        sys.path.append(_p)


import math
import numpy as np

import concourse.bass as bass
import concourse.mybir as mybir
import concourse.tile as tile
from concourse import bacc
from concourse.masks import make_identity

F32 = mybir.dt.float32
F16 = mybir.dt.float16
I32 = mybir.dt.int32
I16 = mybir.dt.int16

N_CORES = 8
D = 256
HC = 256
H = 8
CH = 32
OUT = 32
P = 128
RW1 = 384        # layer-1 table row width (fp16) = 768B
RW2 = 128        # layer-2 table row width (fp16) = 256B
SHIFT1 = 6.0
SHIFT2 = 0.0
SENT_AS = -30000.0


def fold_weights(W1, a_src1, a_dst1, W2, a_src2, a_dst2):
    W1 = np.asarray(W1, np.float32)
    a_src1 = np.asarray(a_src1, np.float32)
    a_dst1 = np.asarray(a_dst1, np.float32)
    A_src = np.zeros((HC, H), np.float32)
    A_dst = np.zeros((HC, H), np.float32)
    for h in range(H):
        A_src[h * CH:(h + 1) * CH, h] = a_src1[h]
        A_dst[h * CH:(h + 1) * CH, h] = a_dst1[h]
    W1ext = np.concatenate([W1, W1 @ A_src, W1 @ A_dst], axis=1)  # [256, 272]
    W2 = np.asarray(W2, np.float32)
    W2ext = np.concatenate(
        [W2, W2 @ np.asarray(a_src2, np.float32).T,
         W2 @ np.asarray(a_dst2, np.float32).T], axis=1)          # [256, 34]
    return W1ext.astype(np.float16), W2ext.astype(np.float16)


def make_cfg(n_valid, npc, n_cores=N_CORES, losplit=None):
    ngroups = math.ceil(npc / P)
    sent = n_valid
    npad = P * math.ceil((n_valid + 1) / P)
    ntiles = npad // P
    if losplit is None:
        if npad > 32512:
            losplit = 32512
        else:
            losplit = max(P, (ntiles // 2) * P)
    assert losplit % P == 0 and 0 < losplit < npad
    lo_tiles = losplit // P
    lo_rows = losplit + P          # + sentinel row block
    hi_rows = npad - losplit       # global sentinel lives here: sent - losplit
    assert losplit <= 32767 and hi_rows <= 32767
    return dict(
        n_valid=n_valid, npc=npc, n_cores=n_cores, ngroups=ngroups,
        sent=sent, npad=npad, ntiles=ntiles, losplit=losplit,
        lo_tiles=lo_tiles, lo_rows=lo_rows, hi_rows=hi_rows,
        lo_sent=losplit, hi_sent=sent - losplit,
    )


def build_edge_tables(src, dst, cfg, eb_lo=None, eb_hi=None):
    """Per-core gather/index tables.

    Returns a list (per core) of dicts with:
      lo_idx  [128, ngroups*EB_LO*8]  int16 (dma_gather wrapped layout)
      hi_idx  [128, ngroups*EB_HI*8]  int16
      dstl_cols [128, ngroups*(EB_LO+EB_HI)] float32
      nodeidx [128, ngroups] int32 (global node per (slot, group))
    """
    n_cores, npc, ngroups = cfg["n_cores"], cfg["npc"], cfg["ngroups"]
    losplit, sent = cfg["losplit"], cfg["sent"]
    lo_sent, hi_sent = cfg["lo_sent"], cfg["hi_sent"]

    src = np.asarray(src, np.int64)
    dst = np.asarray(dst, np.int64)
    core = dst // npc
    per_core = []
    max_lo = max_hi = 0
    for k in range(n_cores):
        m = core == k
        s_k = src[m]
        dl = dst[m] - k * npc
        g_k = dl // P
        islo = s_k < losplit
        cnt_lo = np.bincount(g_k[islo], minlength=ngroups)
        cnt_hi = np.bincount(g_k[~islo], minlength=ngroups)
        max_lo = max(max_lo, int(cnt_lo.max()))
        max_hi = max(max_hi, int(cnt_hi.max()))
        per_core.append((s_k, dl, g_k, islo))
    if eb_lo is None:
        eb_lo = max(1, math.ceil(max_lo / P))
    if eb_hi is None:
        eb_hi = max(1, math.ceil(max_hi / P))
    assert max_lo <= eb_lo * P and max_hi <= eb_hi * P
    ebt = eb_lo + eb_hi

    def wrap16(arr):
        # dma_gather index layout: ordinal i -> [i % 16, i // 16], x8 rows
        n = arr.size
        return np.tile(arr.reshape(n // 16, 16).T, (8, 1)).astype(np.int16)

    tables = []
    for k in range(n_cores):
        s_k, dl, g_k, islo = per_core[k]
        lo_a = np.full((ngroups, eb_lo * P), lo_sent, np.int64)
        hi_a = np.full((ngroups, eb_hi * P), hi_sent, np.int64)
        dstl_a = np.zeros((ngroups, ebt * P), np.int64)
        for g in range(ngroups):
            mg = g_k == g
            m_lo = mg & islo
            m_hi = mg & ~islo
            nlo = int(m_lo.sum())
            nhi = int(m_hi.sum())
            lo_a[g, :nlo] = s_k[m_lo]
            hi_a[g, :nhi] = s_k[m_hi] - losplit
            dstl_a[g, :nlo] = dl[m_lo] - g * P
            dstl_a[g, eb_lo * P:eb_lo * P + nhi] = dl[m_hi] - g * P
        lo_idx = np.concatenate([wrap16(lo_a[g]) for g in range(ngroups)], axis=1)
        hi_idx = np.concatenate([wrap16(hi_a[g]) for g in range(ngroups)], axis=1)
        dstl_cols = np.ascontiguousarray(
            dstl_a.reshape(ngroups, ebt, P).transpose(2, 0, 1)
            .reshape(P, ngroups * ebt)).astype(np.float32)
        nodeidx = (np.arange(P)[:, None] + P * np.arange(ngroups)[None, :]
                   + k * npc)
        nodeidx = np.where(nodeidx < (k + 1) * npc, nodeidx, sent)
        tables.append(dict(
            lo_idx=lo_idx, hi_idx=hi_idx, dstl_cols=dstl_cols,
            nodeidx=nodeidx.astype(np.int32),
        ))
    return tables, eb_lo, eb_hi


# --------------------------------------------------------------------------
# launch A: layer 1
# --------------------------------------------------------------------------

def build_launch_a(cfg, eb_lo, eb_hi, num_devices=N_CORES):
    n_valid, npad, ntiles = cfg["n_valid"], cfg["npad"], cfg["ntiles"]
    npc, ngroups = cfg["npc"], cfg["ngroups"]
    lo_tiles = cfg["lo_tiles"]
    lo_rows, hi_rows = cfg["lo_rows"], cfg["hi_rows"]
    lo_sent, hi_sent = cfg["lo_sent"], cfg["hi_sent"]
    ebt = eb_lo + eb_hi
    ncols = ngroups * ebt

    nc = bacc.Bacc("TRN2", target_bir_lowering=False, debug=False,
                   num_devices=num_devices)
    x_ap = nc.dram_tensor("x", [n_valid, D], F32, kind="ExternalInput").ap()
    w1_ap = nc.dram_tensor("w1ext", [D, HC + 2 * H], F16, kind="ExternalInput").ap()
    b1_ap = nc.dram_tensor("b1", [HC], F32, kind="ExternalInput").ap()
    lo_ap = nc.dram_tensor("lo_idx", [P, ngroups * eb_lo * 8], I16,
                           kind="ExternalInput").ap()
    hi_ap = nc.dram_tensor("hi_idx", [P, ngroups * eb_hi * 8], I16,
                           kind="ExternalInput").ap()
    dstl_ap = nc.dram_tensor("dstl_cols", [P, ncols], F32, kind="ExternalInput").ap()
    ni_ap = nc.dram_tensor("nodeidx", [P, ngroups], I32, kind="ExternalInput").ap()
    out_ap = nc.dram_tensor("hl1T", [HC, npc], F16, kind="ExternalOutput").ap()

    hx_lo = nc.dram_tensor("hx_lo", [lo_rows, RW1], F16).ap()
    hx_hi = nc.dram_tensor("hx_hi", [hi_rows, RW1], F16).ap()
    ad1_g = nc.dram_tensor("ad1_glob", [npad, H], F16).ap()

    TB = 4

    with tile.TileContext(nc) as tc:
        with tc.tile_pool(name="const", bufs=1) as cpool:
            ident16 = cpool.tile([P, P], F16)
            make_identity(nc, ident16[:])
            iota_f = cpool.tile([P, P], F32)
            nc.gpsimd.iota(iota_f[:], pattern=[[1, P]], base=0,
                           channel_multiplier=0,
                           allow_small_or_imprecise_dtypes=True)
            w1_sb = cpool.tile([P, 2, HC + 2 * H], F16)
            nc.sync.dma_start(out=w1_sb[:, 0, :], in_=w1_ap[0:P, :])
            nc.sync.dma_start(out=w1_sb[:, 1, :], in_=w1_ap[P:2 * P, :])
            b1bc = cpool.tile([P, HC], F32)
            nc.sync.dma_start(out=b1bc[:], in_=b1_ap[None, :].to_broadcast([P, HC]))
            lo_sb = cpool.tile([P, ngroups * eb_lo * 8], I16)
            nc.sync.dma_start(out=lo_sb[:], in_=lo_ap[:])
            hi_sb = cpool.tile([P, ngroups * eb_hi * 8], I16)
            nc.sync.dma_start(out=hi_sb[:], in_=hi_ap[:])
            dstl = cpool.tile([P, ncols], F32)
            nc.sync.dma_start(out=dstl[:], in_=dstl_ap[:])
            nodei = cpool.tile([P, ngroups], I32)
            nc.sync.dma_start(out=nodei[:], in_=ni_ap[:])
            nshift = cpool.tile([P, 1], F32)
            nc.gpsimd.memset(nshift[:], -SHIFT1)

            # ---------------- phase A: hx tables = x @ W1ext ---------------
            with (
                tc.tile_pool(name="pa_sbuf", bufs=3) as spool,
                tc.tile_pool(name="pa_out", bufs=3) as opool,
                tc.tile_pool(name="pa_psum", bufs=2, space="PSUM") as pps,
                tc.tile_pool(name="pa_psumT", bufs=3, space="PSUM") as ppsT,
            ):
                for t0 in range(0, ntiles, TB):
                    tb = min(TB, ntiles - t0)
                    full = (t0 + tb) * P <= n_valid
                    x16 = spool.tile([P, TB, D], F16, tag="x16")
                    if full:
                        nc.gpsimd.dma_start(
                            out=x16[:, :tb, :],
                            in_=x_ap[t0 * P:(t0 + tb) * P, :].rearrange(
                                "(t p) c -> p t c", p=P))
                    else:
                        nc.gpsimd.memset(x16[:], 0.0)
                        for ti in range(tb):
                            r0 = (t0 + ti) * P
                            v = min(P, n_valid - r0)
                            if v > 0:
                                nc.gpsimd.dma_start(
                                    out=x16[:v, ti, :], in_=x_ap[r0:r0 + v, :])
                    hx4 = opool.tile([P, TB, RW1], F16, tag="hx4")
                    nc.vector.memset(hx4[:, :, HC + H:], 0.0)
                    ad4 = opool.tile([P, TB, H], F16, tag="ad4")
                    for ti in range(tb):
                        hx_ps = pps.tile([P, HC + 2 * H], F32, tag="hx_ps")
                        for kk in range(2):
                            xT_ps = ppsT.tile([P, P], F16, tag="xT_ps")
                            nc.tensor.transpose(
                                xT_ps[:], x16[:, ti, kk * P:(kk + 1) * P],
                                ident16[:])
                            xT_sb = spool.tile([P, P], F16, tag="xT_sb")
                            nc.vector.tensor_copy(xT_sb[:], xT_ps[:])
                            nc.tensor.matmul(hx_ps[:], lhsT=xT_sb[:],
                                             rhs=w1_sb[:, kk, :],
                                             start=(kk == 0), stop=(kk == 1))
                        nc.scalar.copy(hx4[:, ti, :HC + H], hx_ps[:, :HC + H])
                        nc.scalar.copy(ad4[:, ti, :], hx_ps[:, HC + H:])
                    for ti in range(tb):
                        t = t0 + ti
                        if t < lo_tiles:
                            dst_tab, r0 = hx_lo, t * P
                        else:
                            dst_tab, r0 = hx_hi, (t - lo_tiles) * P
                        nc.sync.dma_start(out=dst_tab[r0:r0 + P, :],
                                          in_=hx4[:, ti, :])
                        nc.sync.dma_start(out=ad1_g[t * P:(t + 1) * P, :],
                                          in_=ad4[:, ti, :])
                # sentinels (write the whole padding block so the tables
                # are fully initialized)
                sent_row = opool.tile([P, RW1], F16, tag="sent_row")
                nc.vector.memset(sent_row[:], 0.0)
                nc.vector.memset(sent_row[:, HC:HC + H], SENT_AS)
                nc.sync.dma_start(out=hx_lo[lo_sent:lo_sent + P, :],
                                  in_=sent_row[:])
                nc.sync.dma_start(out=hx_hi[hi_sent:hi_sent + 1, HC:HC + H],
                                  in_=sent_row[0:1, HC:HC + H])

            # ---------------- phase B: edge aggregation --------------------
            with (
                tc.tile_pool(name="pb_gather", bufs=2) as gpool,
                tc.tile_pool(name="pb_work", bufs=4) as wpool,
                tc.tile_pool(name="pb_ep", bufs=2) as epool,
                tc.tile_pool(name="pb_psum", bufs=2, space="PSUM") as upps,
                tc.tile_pool(name="pb_psmm", bufs=2, space="PSUM") as mpps,
                tc.tile_pool(name="pb_psumT", bufs=2, space="PSUM") as tpps,
            ):
                for g in range(ngroups):
                    glo = gpool.tile([P, eb_lo * RW1], F16, tag="glo")
                    nc.gpsimd.dma_gather(
                        out_ap=glo[:].rearrange("p (b w) -> p b w", w=RW1),
                        in_ap=hx_lo[:],
                        idxs_ap=lo_sb[:, g * eb_lo * 8:(g + 1) * eb_lo * 8],
                        num_idxs=eb_lo * P, num_idxs_reg=eb_lo * P,
                        elem_size=RW1, single_packet=False)
                    ghi = gpool.tile([P, eb_hi * RW1], F16, tag="ghi")
                    nc.gpsimd.dma_gather(
                        out_ap=ghi[:].rearrange("p (b w) -> p b w", w=RW1),
                        in_ap=hx_hi[:],
                        idxs_ap=hi_sb[:, g * eb_hi * 8:(g + 1) * eb_hi * 8],
                        num_idxs=eb_hi * P, num_idxs_reg=eb_hi * P,
                        elem_size=RW1, single_packet=False)
                    adg = gpool.tile([P, H], F16, tag="adg")
                    nc.gpsimd.indirect_dma_start(
                        out=adg[:], out_offset=None, in_=ad1_g[:],
                        in_offset=bass.IndirectOffsetOnAxis(
                            ap=nodei[:, g:g + 1], axis=0))

                    u_ps = upps.tile([P, HC + H], F32, tag="u_ps")
                    for j in range(ebt):
                        c = g * ebt + j
                        if j < eb_lo:
                            gsl = glo[:, j * RW1:(j + 1) * RW1]
                        else:
                            jj = j - eb_lo
                            gsl = ghi[:, jj * RW1:(jj + 1) * RW1]
                        m_sb = wpool.tile([P, P], F16, tag="m_sb")
                        nc.vector.tensor_scalar(
                            out=m_sb[:], in0=iota_f[:],
                            scalar1=dstl[:, c:c + 1], scalar2=None,
                            op0=mybir.AluOpType.is_equal)
                        mT_ps = tpps.tile([P, P], F16, tag="T_ps")
                        nc.tensor.transpose(mT_ps[:], m_sb[:], ident16[:])
                        mT_sb = wpool.tile([P, P], F16, tag="mT_sb")
                        nc.vector.tensor_copy(mT_sb[:], mT_ps[:])
                        ad_ps = mpps.tile([P, H], F32, tag="ad_ps")
                        nc.tensor.matmul(ad_ps[:], lhsT=mT_sb[:], rhs=adg[:],
                                         start=True, stop=True)
                        z = wpool.tile([P, H], F32, tag="z")
                        nc.vector.tensor_tensor(
                            out=z[:], in0=gsl[:, HC:HC + H], in1=ad_ps[:],
                            op=mybir.AluOpType.add)
                        z2 = wpool.tile([P, H], F32, tag="z2")
                        nc.vector.tensor_scalar_mul(z2[:], z[:], 0.2)
                        lr = wpool.tile([P, H], F32, tag="lr")
                        nc.vector.tensor_tensor(out=lr[:], in0=z[:], in1=z2[:],
                                                op=mybir.AluOpType.max)
                        g_sb = wpool.tile([P, HC + H], F16, tag="g_sb")
                        nc.scalar.activation(
                            out=g_sb[:, HC:], in_=lr[:],
                            func=mybir.ActivationFunctionType.Exp,
                            bias=nshift[:])
                        nc.vector.tensor_tensor(
                            out=g_sb[:, :HC].rearrange("p (h c) -> p h c", c=CH),
                            in0=gsl[:, :HC].rearrange("p (h c) -> p h c", c=CH),
                            in1=g_sb[:, HC:][:, :, None].to_broadcast([P, H, CH]),
                            op=mybir.AluOpType.mult)
                        nc.tensor.matmul(u_ps[:], lhsT=m_sb[:], rhs=g_sb[:],
                                         start=(j == 0), stop=(j == ebt - 1))

                    s_sb = epool.tile([P, H], F32, tag="s_sb")
                    nc.vector.tensor_scalar_add(s_sb[:], u_ps[:, HC:], 1e-16)
                    r_sb = epool.tile([P, H], F32, tag="r_sb")
                    nc.vector.reciprocal(r_sb[:], s_sb[:])
                    zt = epool.tile([P, HC], F32, tag="zt")
                    nc.vector.tensor_tensor(
                        out=zt[:].rearrange("p (h c) -> p h c", c=CH),
                        in0=u_ps[:, :HC].rearrange("p (h c) -> p h c", c=CH),
                        in1=r_sb[:][:, :, None].to_broadcast([P, H, CH]),
                        op=mybir.AluOpType.mult)
                    zb = epool.tile([P, HC], F32, tag="zb")
                    nc.vector.tensor_tensor(out=zb[:], in0=zt[:], in1=b1bc[:],
                                            op=mybir.AluOpType.add)
                    t1 = epool.tile([P, HC], F32, tag="t1")
                    nc.vector.tensor_scalar(out=t1[:], in0=zb[:], scalar1=0.0,
                                            scalar2=None,
                                            op0=mybir.AluOpType.min)
                    t2 = epool.tile([P, HC], F32, tag="t2")
                    nc.scalar.activation(out=t2[:], in_=t1[:],
                                         func=mybir.ActivationFunctionType.Exp)
                    t3 = epool.tile([P, HC], F32, tag="t3")
                    nc.vector.tensor_scalar_add(t3[:], t2[:], -1.0)
                    h16 = epool.tile([P, HC], F16, tag="h16")
                    nc.vector.tensor_tensor(out=h16[:], in0=zb[:], in1=t3[:],
                                            op=mybir.AluOpType.max)
                    cols = min(P, npc - g * P)
                    for kk in range(2):
                        hT_ps = tpps.tile([P, P], F16, tag="T_ps")
                        nc.tensor.transpose(hT_ps[:],
                                            h16[:, kk * P:(kk + 1) * P],
                                            ident16[:])
                        hT_sb = epool.tile([P, P], F16, tag="hT_sb")
                        nc.vector.tensor_copy(hT_sb[:], hT_ps[:])
                        nc.sync.dma_start(
                            out=out_ap[kk * P:(kk + 1) * P,
                                       g * P:g * P + cols],
                            in_=hT_sb[:, :cols])
    nc.compile()
    return nc


# --------------------------------------------------------------------------
# launch B: layer 2
# --------------------------------------------------------------------------

def build_launch_b(cfg, eb_lo, eb_hi, num_devices=N_CORES):
    npad, ntiles = cfg["npad"], cfg["ntiles"]
    npc, ngroups = cfg["npc"], cfg["ngroups"]
    lo_tiles = cfg["lo_tiles"]
    lo_rows, hi_rows = cfg["lo_rows"], cfg["hi_rows"]
    lo_sent, hi_sent = cfg["lo_sent"], cfg["hi_sent"]
    ebt = eb_lo + eb_hi
    ncols = ngroups * ebt
    W2C = OUT + 2

    nc = bacc.Bacc("TRN2", target_bir_lowering=False, debug=False,
                   num_devices=num_devices)
    h_ap = nc.dram_tensor("hl1T", [HC, npad], F16, kind="ExternalInput").ap()
    w2_ap = nc.dram_tensor("w2ext", [HC, W2C], F16, kind="ExternalInput").ap()
    b2_ap = nc.dram_tensor("b2", [OUT], F32, kind="ExternalInput").ap()
    lo_ap = nc.dram_tensor("lo_idx", [P, ngroups * eb_lo * 8], I16,
                           kind="ExternalInput").ap()
    hi_ap = nc.dram_tensor("hi_idx", [P, ngroups * eb_hi * 8], I16,
                           kind="ExternalInput").ap()
    dstl_ap = nc.dram_tensor("dstl_cols", [P, ncols], F32, kind="ExternalInput").ap()
    ni_ap = nc.dram_tensor("nodeidx", [P, ngroups], I32, kind="ExternalInput").ap()
    y_ap = nc.dram_tensor("y", [npc, OUT], F32, kind="ExternalOutput").ap()

    hx2_lo = nc.dram_tensor("hx2_lo", [lo_rows, RW2], F16).ap()
    hx2_hi = nc.dram_tensor("hx2_hi", [hi_rows, RW2], F16).ap()
    ad2_g = nc.dram_tensor("ad2_glob", [npad, 1], F16).ap()

    TB = 4

    with tile.TileContext(nc) as tc:
        with tc.tile_pool(name="const", bufs=1) as cpool:
            ident32 = cpool.tile([P, P], F32)
            make_identity(nc, ident32[:])
            iota_f = cpool.tile([P, P], F32)
            nc.gpsimd.iota(iota_f[:], pattern=[[1, P]], base=0,
                           channel_multiplier=0,
                           allow_small_or_imprecise_dtypes=True)
            w2_sb = cpool.tile([P, 2, W2C], F16)
            nc.sync.dma_start(out=w2_sb[:, 0, :], in_=w2_ap[0:P, :])
            nc.sync.dma_start(out=w2_sb[:, 1, :], in_=w2_ap[P:2 * P, :])
            b2bc = cpool.tile([P, OUT], F32)
            nc.sync.dma_start(out=b2bc[:], in_=b2_ap[None, :].to_broadcast([P, OUT]))
            lo_sb = cpool.tile([P, ngroups * eb_lo * 8], I16)
            nc.sync.dma_start(out=lo_sb[:], in_=lo_ap[:])
            hi_sb = cpool.tile([P, ngroups * eb_hi * 8], I16)
            nc.sync.dma_start(out=hi_sb[:], in_=hi_ap[:])
            dstl = cpool.tile([P, ncols], F32)
            nc.sync.dma_start(out=dstl[:], in_=dstl_ap[:])
            nodei = cpool.tile([P, ngroups], I32)
            nc.sync.dma_start(out=nodei[:], in_=ni_ap[:])

            # ---------------- phase A: hx2 tables = hl1 @ W2ext ------------
            with (
                tc.tile_pool(name="pa_sbuf", bufs=3) as spool,
                tc.tile_pool(name="pa_out", bufs=3) as opool,
                tc.tile_pool(name="pa_psum", bufs=4, space="PSUM") as pps,
            ):
                for t0 in range(0, ntiles, TB):
                    tb = min(TB, ntiles - t0)
                    hT = spool.tile([P, 2, TB * P], F16, tag="hT")
                    for kk in range(2):
                        nc.sync.dma_start(
                            out=hT[:, kk, :tb * P],
                            in_=h_ap[kk * P:(kk + 1) * P,
                                     t0 * P:(t0 + tb) * P])
                    h24 = opool.tile([P, TB, RW2], F16, tag="h24")
                    nc.vector.memset(h24[:, :, W2C:], 0.0)
                    for ti in range(tb):
                        h2_ps = pps.tile([P, W2C], F32, tag="h2_ps")
                        for kk in range(2):
                            nc.tensor.matmul(
                                h2_ps[:],
                                lhsT=hT[:, kk, ti * P:(ti + 1) * P],
                                rhs=w2_sb[:, kk, :],
                                start=(kk == 0), stop=(kk == 1))
                        nc.scalar.copy(h24[:, ti, :W2C], h2_ps[:])
                    for ti in range(tb):
                        t = t0 + ti
                        if t < lo_tiles:
                            dst_tab, r0 = hx2_lo, t * P
                        else:
                            dst_tab, r0 = hx2_hi, (t - lo_tiles) * P
                        nc.sync.dma_start(out=dst_tab[r0:r0 + P, :],
                                          in_=h24[:, ti, :])
                        nc.sync.dma_start(out=ad2_g[t * P:(t + 1) * P, :],
                                          in_=h24[:, ti, OUT + 1:W2C])
                sent_row = opool.tile([P, RW2], F16, tag="sent_row")
                nc.vector.memset(sent_row[:], 0.0)
                nc.vector.memset(sent_row[:, OUT:OUT + 1], SENT_AS)
                nc.sync.dma_start(out=hx2_lo[lo_sent:lo_sent + P, :],
                                  in_=sent_row[:])
                nc.sync.dma_start(out=hx2_hi[hi_sent:hi_sent + 1, OUT:OUT + 1],
                                  in_=sent_row[0:1, OUT:OUT + 1])

            # ---------------- phase B: edge aggregation --------------------
            with (
                tc.tile_pool(name="pb_gather", bufs=2) as gpool,
                tc.tile_pool(name="pb_work", bufs=4) as wpool,
                tc.tile_pool(name="pb_ep", bufs=2) as epool,
                tc.tile_pool(name="pb_psum", bufs=2, space="PSUM") as upps,
                tc.tile_pool(name="pb_psmm", bufs=2, space="PSUM") as mpps,
                tc.tile_pool(name="pb_psumT", bufs=2, space="PSUM") as tpps,
            ):
                for g in range(ngroups):
                    glo = gpool.tile([P, eb_lo * RW2], F16, tag="glo")
                    nc.gpsimd.dma_gather(
                        out_ap=glo[:].rearrange("p (b w) -> p b w", w=RW2),
                        in_ap=hx2_lo[:],
                        idxs_ap=lo_sb[:, g * eb_lo * 8:(g + 1) * eb_lo * 8],
                        num_idxs=eb_lo * P, num_idxs_reg=eb_lo * P,
                        elem_size=RW2, single_packet=False)
                    ghi = gpool.tile([P, eb_hi * RW2], F16, tag="ghi")
                    nc.gpsimd.dma_gather(
                        out_ap=ghi[:].rearrange("p (b w) -> p b w", w=RW2),
                        in_ap=hx2_hi[:],
                        idxs_ap=hi_sb[:, g * eb_hi * 8:(g + 1) * eb_hi * 8],
                        num_idxs=eb_hi * P, num_idxs_reg=eb_hi * P,
                        elem_size=RW2, single_packet=False)
                    adg16 = gpool.tile([P, 1], F16, tag="adg16")
                    nc.gpsimd.indirect_dma_start(
                        out=adg16[:], out_offset=None, in_=ad2_g[:],
                        in_offset=bass.IndirectOffsetOnAxis(
                            ap=nodei[:, g:g + 1], axis=0))
                    adg = gpool.tile([P, 1], F32, tag="adg")
                    nc.vector.tensor_copy(adg[:], adg16[:])

                    u_ps = upps.tile([P, OUT + 1], F32, tag="u_ps")
                    for j in range(ebt):
                        c = g * ebt + j
                        if j < eb_lo:
                            gsl = glo[:, j * RW2:(j + 1) * RW2]
                        else:
                            jj = j - eb_lo
                            gsl = ghi[:, jj * RW2:(jj + 1) * RW2]
                        m_sb = wpool.tile([P, P], F32, tag="m_sb")
                        nc.vector.tensor_scalar(
                            out=m_sb[:], in0=iota_f[:],
                            scalar1=dstl[:, c:c + 1], scalar2=None,
                            op0=mybir.AluOpType.is_equal)
                        mT_ps = tpps.tile([P, P], F32, tag="T_ps")
                        nc.tensor.transpose(mT_ps[:], m_sb[:], ident32[:])
                        mT_sb = wpool.tile([P, P], F32, tag="mT_sb")
                        nc.vector.tensor_copy(mT_sb[:], mT_ps[:])
                        ad_ps = mpps.tile([P, 1], F32, tag="ad_ps")
                        nc.tensor.matmul(ad_ps[:], lhsT=mT_sb[:], rhs=adg[:],
                                         start=True, stop=True)
                        z = wpool.tile([P, 1], F32, tag="z")
                        nc.vector.tensor_tensor(
                            out=z[:], in0=gsl[:, OUT:OUT + 1], in1=ad_ps[:],
                            op=mybir.AluOpType.add)
                        z2 = wpool.tile([P, 1], F32, tag="z2")
                        nc.vector.tensor_scalar_mul(z2[:], z[:], 0.2)
                        lr = wpool.tile([P, 1], F32, tag="lr")
                        nc.vector.tensor_tensor(out=lr[:], in0=z[:], in1=z2[:],
                                                op=mybir.AluOpType.max)
                        g_sb = wpool.tile([P, OUT + 1], F32, tag="g_sb")
                        nc.scalar.activation(
                            out=g_sb[:, OUT:], in_=lr[:],
                            func=mybir.ActivationFunctionType.Exp, bias=0.0)
                        nc.vector.tensor_scalar(
                            out=g_sb[:, :OUT], in0=gsl[:, :OUT],
                            scalar1=g_sb[:, OUT:OUT + 1], scalar2=None,
                            op0=mybir.AluOpType.mult)
                        nc.tensor.matmul(u_ps[:], lhsT=m_sb[:], rhs=g_sb[:],
                                         start=(j == 0), stop=(j == ebt - 1))

                    s_sb = epool.tile([P, 1], F32, tag="s_sb")
                    nc.vector.tensor_scalar_add(s_sb[:], u_ps[:, OUT:], 1e-16)
                    r_sb = epool.tile([P, 1], F32, tag="r_sb")
                    nc.vector.reciprocal(r_sb[:], s_sb[:])
                    y_sb = epool.tile([P, OUT], F32, tag="y_sb")
                    nc.vector.tensor_scalar(
                        out=y_sb[:], in0=u_ps[:, :OUT],
                        scalar1=r_sb[:, 0:1], scalar2=None,
                        op0=mybir.AluOpType.mult)
                    yb = epool.tile([P, OUT], F32, tag="yb")
                    nc.vector.tensor_tensor(out=yb[:], in0=y_sb[:], in1=b2bc[:],
                                            op=mybir.AluOpType.add)
                    rows = min(P, npc - g * P)
                    nc.sync.dma_start(out=y_ap[g * P:g * P + rows, :],
                                      in_=yb[:rows, :])
    nc.compile()
    return nc


# --------------------------------------------------------------------------
# numpy emulation of the on-device algorithm (for logic validation)
# --------------------------------------------------------------------------

def emulate(x, W1ext, b1, W2ext, b2, cfg, tables, eb_lo, eb_hi):
    f16 = np.float16
    n_valid, npc, npad = cfg["n_valid"], cfg["npc"], cfg["npad"]
    ngroups, sent = cfg["ngroups"], cfg["sent"]
    n_cores, losplit = cfg["n_cores"], cfg["losplit"]
    ebt = eb_lo + eb_hi

    xpad = np.zeros((npad, D), np.float32)
    xpad[:n_valid] = x
    hx = (xpad.astype(f16).astype(np.float32) @
          W1ext.astype(np.float32)).astype(f16)     # [npad, 272]
    hxrow = np.zeros((npad + 1, RW1), f16)          # +1 = lo sentinel
    hxrow[:npad, :HC + H] = hx[:, :HC + H]
    hxrow[npad, HC:HC + H] = f16(SENT_AS)           # lo sentinel row
    hxrow[sent, HC:HC + H] = f16(SENT_AS)
    ad1 = hx[:, HC + H:].copy()
    ad1 = np.concatenate([ad1, np.zeros((1, H), f16)], axis=0)

    def row_of(tab, is_lo, idx):
        return np.where(is_lo, idx, np.minimum(idx + losplit, npad))

    hl1T = np.zeros((HC, npad), f16)
    for k in range(n_cores):
        tb = tables[k]
        lo_flat = tb["lo_idx"][:16].T.reshape(ngroups, eb_lo * P)
        hi_flat = tb["hi_idx"][:16].T.reshape(ngroups, eb_hi * P)
        for g in range(ngroups):
            U = np.zeros((P, HC + H), np.float32)
            adg = ad1[np.minimum(tb["nodeidx"][:, g], npad)].astype(np.float32)
            for j in range(ebt):
                if j < eb_lo:
                    idx = lo_flat[g, j * P:(j + 1) * P]
                    rows_idx = np.where(idx == cfg["lo_sent"], npad, idx)
                else:
                    jj = j - eb_lo
                    idx = hi_flat[g, jj * P:(jj + 1) * P]
                    rows_idx = idx + losplit
                rows = hxrow[rows_idx].astype(np.float32)
                dstl = tb["dstl_cols"][:, g * ebt + j]
                M = (np.arange(P)[None, :] == dstl[:, None]).astype(np.float32)
                ad_e = M @ adg
                z = rows[:, HC:HC + H] + ad_e
                lr = np.maximum(z, 0.2 * z)
                ex = np.exp(lr - SHIFT1).astype(f16).astype(np.float32)
                G = np.concatenate(
                    [(rows[:, :HC].reshape(P, H, CH) *
                      ex[:, :, None]).reshape(P, HC), ex], axis=1).astype(f16)
                U += M.T @ G.astype(np.float32)
            s = U[:, HC:] + 1e-16
            z = (U[:, :HC].reshape(P, H, CH) / s[:, :, None]).reshape(P, HC) + b1
            h1e = np.maximum(z, np.exp(np.minimum(z, 0)) - 1).astype(f16)
            cols = min(P, npc - g * P)
            hl1T[:, k * npc + g * P:k * npc + g * P + cols] = h1e.T[:, :cols]

    hx2 = (hl1T.T.astype(np.float32) @ W2ext.astype(np.float32)).astype(f16)
    hx2row = np.zeros((npad + 1, OUT + 2), f16)
    hx2row[:npad] = hx2
    hx2row[npad, OUT] = f16(SENT_AS)
    hx2row[sent, OUT] = f16(SENT_AS)
    ad2 = np.concatenate([hx2[:, OUT + 1:], np.zeros((1, 1), f16)], axis=0)

    y = np.zeros((n_valid, OUT), np.float32)
    for k in range(n_cores):
        tb = tables[k]
        lo_flat = tb["lo_idx"][:16].T.reshape(ngroups, eb_lo * P)
        hi_flat = tb["hi_idx"][:16].T.reshape(ngroups, eb_hi * P)
        for g in range(ngroups):
            U = np.zeros((P, OUT + 1), np.float32)
            adg = ad2[np.minimum(tb["nodeidx"][:, g], npad)].astype(np.float32)
            for j in range(ebt):
                if j < eb_lo:
                    idx = lo_flat[g, j * P:(j + 1) * P]
                    rows_idx = np.where(idx == cfg["lo_sent"], npad, idx)
                else:
                    jj = j - eb_lo
                    idx = hi_flat[g, jj * P:(jj + 1) * P]
                    rows_idx = idx + losplit
                rows = hx2row[rows_idx].astype(np.float32)
                dstl = tb["dstl_cols"][:, g * ebt + j]
                M = (np.arange(P)[None, :] == dstl[:, None]).astype(np.float32)
                ad_e = (M @ adg)[:, 0]
                z = rows[:, OUT] + ad_e
                lr = np.maximum(z, 0.2 * z)
                ex = np.exp(lr - SHIFT2)
                G = np.concatenate([rows[:, :OUT] * ex[:, None],
                                    ex[:, None]], axis=1)
                U += M.T @ G
            s = U[:, OUT] + 1e-16
            yg = U[:, :OUT] / s[:, None] + b2
            rows_n = min(P, npc - g * P)
            base = k * npc + g * P
            if base < n_valid:
                y[base:base + rows_n] = yg[:rows_n]
    return y


# --------------------------------------------------------------------------
# top-level kernel entry point (full inputs in, full output out)
# --------------------------------------------------------------------------

_CACHE = {}


def kernel(**inputs):
    """Full-input GAT kernel on 8 Trainium2 NeuronCores.

    Takes the unsharded inputs of reference.setup_inputs(), distributes the
    work across 8 cores (dst-node graph partition), and returns the full
    [50000, 32] float32 output.
    """
    from concourse.bass_utils import run_bass_kernel_spmd

    x = np.ascontiguousarray(np.asarray(inputs["x"], np.float32))
    ei = np.asarray(inputs["edge_index"])
    N = x.shape[0]
    npc = N // N_CORES
    assert npc * N_CORES == N

    src = np.concatenate([ei[0].astype(np.int64), np.arange(N, dtype=np.int64)])
    dst = np.concatenate([ei[1].astype(np.int64), np.arange(N, dtype=np.int64)])

    cfg = make_cfg(N, npc)
    tables, eb_lo, eb_hi = build_edge_tables(src, dst, cfg)
    W1ext, W2ext = fold_weights(
        inputs["W1"], inputs["a_src1"], inputs["a_dst1"],
        inputs["W2"], inputs["a_src2"], inputs["a_dst2"])
    b1 = np.asarray(inputs["b1"], np.float32)
    b2 = np.asarray(inputs["b2"], np.float32)

    key = (N, npc, eb_lo, eb_hi)
    if key not in _CACHE:
        _CACHE[key] = (build_launch_a(cfg, eb_lo, eb_hi),
                       build_launch_b(cfg, eb_lo, eb_hi))
    nc_a, nc_b = _CACHE[key]

    in_maps_a = [dict(
        x=x, w1ext=W1ext, b1=b1,
        lo_idx=tables[k]["lo_idx"], hi_idx=tables[k]["hi_idx"],
        dstl_cols=tables[k]["dstl_cols"], nodeidx=tables[k]["nodeidx"],
    ) for k in range(N_CORES)]
    res_a = run_bass_kernel_spmd(nc_a, in_maps_a, core_ids=list(range(N_CORES)))

    hl1T_pad = np.zeros((HC, cfg["npad"]), np.float16)
    hl1T_pad[:, :N] = np.concatenate(
        [res_a.results[k]["hl1T"] for k in range(N_CORES)], axis=1)

    in_maps_b = [dict(
        hl1T=hl1T_pad, w2ext=W2ext, b2=b2,
        lo_idx=tables[k]["lo_idx"], hi_idx=tables[k]["hi_idx"],
        dstl_cols=tables[k]["dstl_cols"], nodeidx=tables[k]["nodeidx"],
    ) for k in range(N_CORES)]
    res_b = run_bass_kernel_spmd(nc_b, in_maps_b, core_ids=list(range(N_CORES)))

    y = np.concatenate([res_b.results[k]["y"][:npc] for k in range(N_CORES)],
                       axis=0)
    return y.astype(np.float32)


# revision 2
# speedup vs baseline: 9.6667x; 9.6667x over previous
"""Two-layer GAT on 8 Trainium2 cores via Bass/Tile.

Strategy (dst-node graph partition, per the sharding hint):
- Nodes are split into 8 contiguous ranges (6250 per core); every edge is
  owned by the core that owns its dst node.
- Launch A (layer 1): each core redundantly computes the dense part
  hx = x @ [W1 | W1@blockdiag(a_src1) | W1@blockdiag(a_dst1)] into fp16
  DRAM tables, then aggregates its own edges with one-hot PE matmuls:
    * node rows are stored in two tables split at LOSPLIT so dma_gather's
      int16 indices stay < 32768; row width 384 fp16 (768B, 256B-aligned).
    * edges are bucketed per (core, 128-dst-node group) and split into
      lo/hi sub-buckets by src row; each padded to EB_LO/EB_HI blocks of
      128 edges (dummy edges point at sentinel rows with a_src = -30000 so
      exp() == 0).
    * per 128-edge block: M[e,j] = (dst_local[e] == j) via DVE is_equal
      against an iota constant; M_T via PE transpose; per-edge a_dst from
      M_T.T @ ad_group (a [128,8] dense gather per group); logits, exp
      (shifted by -6), and one PE matmul U += M.T @ [ex*h | ex] accumulated
      in PSUM over the group's blocks.
    * group epilogue: h1 = elu(U[:,:256]/(U[:,256:264]+1e-16) + b1),
      transposed on PE, written as hl1T [256, 6250] fp16.
- Host: concatenates the 8 hl1T shards (pure data movement).
- Launch B (layer 2): same structure, OUT=32, one head, fp32 edge math,
  hx2 = hl1 @ [W2 | W2@a_src2.T | W2@a_dst2.T] from hl1T tiles (no
  transposes needed).
"""

import sys
for _p in ("/opt/trn_rl_repo",):
    if _p not in sys.path:
        sys.path.append(_p)


import math
import numpy as np

import concourse.bass as bass
import concourse.mybir as mybir
import concourse.tile as tile
from concourse import bacc
from concourse.masks import make_identity

F32 = mybir.dt.float32
F16 = mybir.dt.float16
I32 = mybir.dt.int32
I16 = mybir.dt.int16

N_CORES = 8
D = 256
HC = 256
H = 8
CH = 32
OUT = 32
P = 128
RW1 = 384        # layer-1 table row width (fp16) = 768B
RW2 = 128        # layer-2 table row width (fp16) = 256B
SHIFT1 = 6.0
SHIFT2 = 0.0
SENT_AS = -30000.0


def fold_weights(W1, a_src1, a_dst1, W2, a_src2, a_dst2):
    W1 = np.asarray(W1, np.float32)
    a_src1 = np.asarray(a_src1, np.float32)
    a_dst1 = np.asarray(a_dst1, np.float32)
    A_src = np.zeros((HC, H), np.float32)
    A_dst = np.zeros((HC, H), np.float32)
    for h in range(H):
        A_src[h * CH:(h + 1) * CH, h] = a_src1[h]
        A_dst[h * CH:(h + 1) * CH, h] = a_dst1[h]
    W1ext = np.concatenate([W1, W1 @ A_src, W1 @ A_dst], axis=1)  # [256, 272]
    W2 = np.asarray(W2, np.float32)
    W2ext = np.concatenate(
        [W2, W2 @ np.asarray(a_src2, np.float32).T,
         W2 @ np.asarray(a_dst2, np.float32).T], axis=1)          # [256, 34]
    return W1ext.astype(np.float16), W2ext.astype(np.float16)


def make_cfg(n_valid, npc, n_cores=N_CORES, losplit=None):
    ngroups = math.ceil(npc / P)
    sent = n_valid
    npad = P * math.ceil((n_valid + 1) / P)
    ntiles = npad // P
    if losplit is None:
        if npad > 32512:
            losplit = 32512
        else:
            losplit = max(P, (ntiles // 2) * P)
    assert losplit % P == 0 and 0 < losplit < npad
    lo_tiles = losplit // P
    lo_rows = losplit + P          # + sentinel row block
    hi_rows = npad - losplit       # global sentinel lives here: sent - losplit
    assert losplit <= 32767 and hi_rows <= 32767
    return dict(
        n_valid=n_valid, npc=npc, n_cores=n_cores, ngroups=ngroups,
        sent=sent, npad=npad, ntiles=ntiles, losplit=losplit,
        lo_tiles=lo_tiles, lo_rows=lo_rows, hi_rows=hi_rows,
        lo_sent=losplit, hi_sent=sent - losplit,
    )


def build_edge_tables(src, dst, cfg, eb_lo=None, eb_hi=None):
    """Per-core gather/index tables.

    Returns a list (per core) of dicts with:
      lo_idx  [128, ngroups*EB_LO*8]  int16 (dma_gather wrapped layout)
      hi_idx  [128, ngroups*EB_HI*8]  int16
      dstl_cols [128, ngroups*(EB_LO+EB_HI)] float32
      nodeidx [128, ngroups] int32 (global node per (slot, group))
    """
    n_cores, npc, ngroups = cfg["n_cores"], cfg["npc"], cfg["ngroups"]
    losplit, sent = cfg["losplit"], cfg["sent"]
    lo_sent, hi_sent = cfg["lo_sent"], cfg["hi_sent"]

    src = np.asarray(src, np.int64)
    dst = np.asarray(dst, np.int64)
    core = dst // npc
    per_core = []
    max_lo = max_hi = 0
    for k in range(n_cores):
        m = core == k
        s_k = src[m]
        dl = dst[m] - k * npc
        g_k = dl // P
        islo = s_k < losplit
        cnt_lo = np.bincount(g_k[islo], minlength=ngroups)
        cnt_hi = np.bincount(g_k[~islo], minlength=ngroups)
        max_lo = max(max_lo, int(cnt_lo.max()))
        max_hi = max(max_hi, int(cnt_hi.max()))
        per_core.append((s_k, dl, g_k, islo))
    if eb_lo is None:
        eb_lo = max(1, math.ceil(max_lo / P))
    if eb_hi is None:
        eb_hi = max(1, math.ceil(max_hi / P))
    assert max_lo <= eb_lo * P and max_hi <= eb_hi * P
    ebt = eb_lo + eb_hi

    def wrap16(arr):
        # dma_gather index layout: ordinal i -> [i % 16, i // 16], x8 rows
        n = arr.size
        return np.tile(arr.reshape(n // 16, 16).T, (8, 1)).astype(np.int16)

    tables = []
    for k in range(n_cores):
        s_k, dl, g_k, islo = per_core[k]
        lo_a = np.full((ngroups, eb_lo * P), lo_sent, np.int64)
        hi_a = np.full((ngroups, eb_hi * P), hi_sent, np.int64)
        dstl_a = np.zeros((ngroups, ebt * P), np.int64)
        for g in range(ngroups):
            mg = g_k == g
            m_lo = mg & islo
            m_hi = mg & ~islo
            nlo = int(m_lo.sum())
            nhi = int(m_hi.sum())
            lo_a[g, :nlo] = s_k[m_lo]
            hi_a[g, :nhi] = s_k[m_hi] - losplit
            dstl_a[g, :nlo] = dl[m_lo] - g * P
            dstl_a[g, eb_lo * P:eb_lo * P + nhi] = dl[m_hi] - g * P
        lo_idx = np.concatenate([wrap16(lo_a[g]) for g in range(ngroups)], axis=1)
        hi_idx = np.concatenate([wrap16(hi_a[g]) for g in range(ngroups)], axis=1)
        dstl_cols = np.ascontiguousarray(
            dstl_a.reshape(ngroups, ebt, P).transpose(2, 0, 1)
            .reshape(P, ngroups * ebt)).astype(np.float32)
        nodeidx = (np.arange(P)[:, None] + P * np.arange(ngroups)[None, :]
                   + k * npc)
        nodeidx = np.where(nodeidx < (k + 1) * npc, nodeidx, sent)
        tables.append(dict(
            lo_idx=lo_idx, hi_idx=hi_idx, dstl_cols=dstl_cols,
            nodeidx=nodeidx.astype(np.int32),
        ))
    return tables, eb_lo, eb_hi


# --------------------------------------------------------------------------
# launch A: layer 1
# --------------------------------------------------------------------------

def build_launch_a(cfg, eb_lo, eb_hi, num_devices=N_CORES):
    n_valid, npad, ntiles = cfg["n_valid"], cfg["npad"], cfg["ntiles"]
    npc, ngroups = cfg["npc"], cfg["ngroups"]
    lo_tiles = cfg["lo_tiles"]
    lo_rows, hi_rows = cfg["lo_rows"], cfg["hi_rows"]
    lo_sent, hi_sent = cfg["lo_sent"], cfg["hi_sent"]
    ebt = eb_lo + eb_hi
    ncols = ngroups * ebt

    nc = bacc.Bacc("TRN2", target_bir_lowering=False, debug=False,
                   num_devices=num_devices)
    x_ap = nc.dram_tensor("x", [n_valid, D], F32, kind="ExternalInput").ap()
    w1_ap = nc.dram_tensor("w1ext", [D, HC + 2 * H], F16, kind="ExternalInput").ap()
    b1_ap = nc.dram_tensor("b1", [HC], F32, kind="ExternalInput").ap()
    lo_ap = nc.dram_tensor("lo_idx", [P, ngroups * eb_lo * 8], I16,
                           kind="ExternalInput").ap()
    hi_ap = nc.dram_tensor("hi_idx", [P, ngroups * eb_hi * 8], I16,
                           kind="ExternalInput").ap()
    dstl_ap = nc.dram_tensor("dstl_cols", [P, ncols], F32, kind="ExternalInput").ap()
    ni_ap = nc.dram_tensor("nodeidx", [P, ngroups], I32, kind="ExternalInput").ap()
    out_ap = nc.dram_tensor("hl1T", [HC, npc], F16, kind="ExternalOutput").ap()

    hx_lo = nc.dram_tensor("hx_lo", [lo_rows, RW1], F16).ap()
    hx_hi = nc.dram_tensor("hx_hi", [hi_rows, RW1], F16).ap()
    ad1_g = nc.dram_tensor("ad1_glob", [npad, H], F16).ap()

    TB = 4

    with tile.TileContext(nc) as tc:
        with tc.tile_pool(name="const", bufs=1) as cpool:
            ident16 = cpool.tile([P, P], F16)
            make_identity(nc, ident16[:])
            iota_f = cpool.tile([P, P], F32)
            nc.gpsimd.iota(iota_f[:], pattern=[[1, P]], base=0,
                           channel_multiplier=0,
                           allow_small_or_imprecise_dtypes=True)
            w1_sb = cpool.tile([P, 2, HC + 2 * H], F16)
            nc.sync.dma_start(out=w1_sb[:, 0, :], in_=w1_ap[0:P, :])
            nc.sync.dma_start(out=w1_sb[:, 1, :], in_=w1_ap[P:2 * P, :])
            b1bc = cpool.tile([P, HC], F32)
            nc.sync.dma_start(out=b1bc[:], in_=b1_ap[None, :].to_broadcast([P, HC]))
            lo_sb = cpool.tile([P, ngroups * eb_lo * 8], I16)
            nc.sync.dma_start(out=lo_sb[:], in_=lo_ap[:])
            hi_sb = cpool.tile([P, ngroups * eb_hi * 8], I16)
            nc.sync.dma_start(out=hi_sb[:], in_=hi_ap[:])
            dstl = cpool.tile([P, ncols], F32)
            nc.sync.dma_start(out=dstl[:], in_=dstl_ap[:])
            nodei = cpool.tile([P, ngroups], I32)
            nc.sync.dma_start(out=nodei[:], in_=ni_ap[:])
            nshift = cpool.tile([P, 1], F32)
            nc.gpsimd.memset(nshift[:], -SHIFT1)

            # ---------------- phase A: hx tables = x @ W1ext ---------------
            with (
                tc.tile_pool(name="pa_sbuf", bufs=3) as spool,
                tc.tile_pool(name="pa_out", bufs=3) as opool,
                tc.tile_pool(name="pa_psum", bufs=2, space="PSUM") as pps,
                tc.tile_pool(name="pa_psumT", bufs=3, space="PSUM") as ppsT,
            ):
                for t0 in range(0, ntiles, TB):
                    tb = min(TB, ntiles - t0)
                    full = (t0 + tb) * P <= n_valid
                    x16 = spool.tile([P, TB, D], F16, tag="x16")
                    if full:
                        nc.gpsimd.dma_start(
                            out=x16[:, :tb, :],
                            in_=x_ap[t0 * P:(t0 + tb) * P, :].rearrange(
                                "(t p) c -> p t c", p=P))
                    else:
                        nc.gpsimd.memset(x16[:], 0.0)
                        for ti in range(tb):
                            r0 = (t0 + ti) * P
                            v = min(P, n_valid - r0)
                            if v > 0:
                                nc.gpsimd.dma_start(
                                    out=x16[:v, ti, :], in_=x_ap[r0:r0 + v, :])
                    hx4 = opool.tile([P, TB, RW1], F16, tag="hx4")
                    nc.vector.memset(hx4[:, :, HC + H:], 0.0)
                    ad4 = opool.tile([P, TB, H], F16, tag="ad4")
                    for ti in range(tb):
                        hx_ps = pps.tile([P, HC + 2 * H], F32, tag="hx_ps")
                        for kk in range(2):
                            xT_ps = ppsT.tile([P, P], F16, tag="xT_ps")
                            nc.tensor.transpose(
                                xT_ps[:], x16[:, ti, kk * P:(kk + 1) * P],
                                ident16[:])
                            xT_sb = spool.tile([P, P], F16, tag="xT_sb")
                            nc.vector.tensor_copy(xT_sb[:], xT_ps[:])
                            nc.tensor.matmul(hx_ps[:], lhsT=xT_sb[:],
                                             rhs=w1_sb[:, kk, :],
                                             start=(kk == 0), stop=(kk == 1))
                        nc.scalar.copy(hx4[:, ti, :HC + H], hx_ps[:, :HC + H])
                        nc.scalar.copy(ad4[:, ti, :], hx_ps[:, HC + H:])
                    for ti in range(tb):
                        t = t0 + ti
                        if t < lo_tiles:
                            dst_tab, r0 = hx_lo, t * P
                        else:
                            dst_tab, r0 = hx_hi, (t - lo_tiles) * P
                        nc.sync.dma_start(out=dst_tab[r0:r0 + P, :],
                                          in_=hx4[:, ti, :])
                        nc.sync.dma_start(out=ad1_g[t * P:(t + 1) * P, :],
                                          in_=ad4[:, ti, :])
                # sentinels (write the whole padding block so the tables
                # are fully initialized)
                sent_row = opool.tile([P, RW1], F16, tag="sent_row")
                nc.vector.memset(sent_row[:], 0.0)
                nc.vector.memset(sent_row[:, HC:HC + H], SENT_AS)
                nc.sync.dma_start(out=hx_lo[lo_sent:lo_sent + P, :],
                                  in_=sent_row[:])
                nc.sync.dma_start(out=hx_hi[hi_sent:hi_sent + 1, HC:HC + H],
                                  in_=sent_row[0:1, HC:HC + H])

            # ---------------- phase B: edge aggregation --------------------
            with (
                tc.tile_pool(name="pb_gather", bufs=2) as gpool,
                tc.tile_pool(name="pb_work", bufs=4) as wpool,
                tc.tile_pool(name="pb_ep", bufs=2) as epool,
                tc.tile_pool(name="pb_psum", bufs=2, space="PSUM") as upps,
                tc.tile_pool(name="pb_psmm", bufs=2, space="PSUM") as mpps,
                tc.tile_pool(name="pb_psumT", bufs=2, space="PSUM") as tpps,
            ):
                for g in range(ngroups):
                    glo = gpool.tile([P, eb_lo * RW1], F16, tag="glo")
                    nc.gpsimd.dma_gather(
                        out_ap=glo[:].rearrange("p (b w) -> p b w", w=RW1),
                        in_ap=hx_lo[:],
                        idxs_ap=lo_sb[:, g * eb_lo * 8:(g + 1) * eb_lo * 8],
                        num_idxs=eb_lo * P, num_idxs_reg=eb_lo * P,
                        elem_size=RW1, single_packet=False)
                    ghi = gpool.tile([P, eb_hi * RW1], F16, tag="ghi")
                    nc.gpsimd.dma_gather(
                        out_ap=ghi[:].rearrange("p (b w) -> p b w", w=RW1),
                        in_ap=hx_hi[:],
                        idxs_ap=hi_sb[:, g * eb_hi * 8:(g + 1) * eb_hi * 8],
                        num_idxs=eb_hi * P, num_idxs_reg=eb_hi * P,
                        elem_size=RW1, single_packet=False)
                    adg = gpool.tile([P, H], F16, tag="adg")
                    nc.gpsimd.indirect_dma_start(
                        out=adg[:], out_offset=None, in_=ad1_g[:],
                        in_offset=bass.IndirectOffsetOnAxis(
                            ap=nodei[:, g:g + 1], axis=0))

                    u_ps = upps.tile([P, HC + H], F32, tag="u_ps")
                    for j in range(ebt):
                        c = g * ebt + j
                        if j < eb_lo:
                            gsl = glo[:, j * RW1:(j + 1) * RW1]
                        else:
                            jj = j - eb_lo
                            gsl = ghi[:, jj * RW1:(jj + 1) * RW1]
                        m_sb = wpool.tile([P, P], F16, tag="m_sb")
                        nc.vector.tensor_scalar(
                            out=m_sb[:], in0=iota_f[:],
                            scalar1=dstl[:, c:c + 1], scalar2=None,
                            op0=mybir.AluOpType.is_equal)
                        mT_ps = tpps.tile([P, P], F16, tag="T_ps")
                        nc.tensor.transpose(mT_ps[:], m_sb[:], ident16[:])
                        mT_sb = wpool.tile([P, P], F16, tag="mT_sb")
                        nc.vector.tensor_copy(mT_sb[:], mT_ps[:])
                        ad_ps = mpps.tile([P, H], F32, tag="ad_ps")
                        nc.tensor.matmul(ad_ps[:], lhsT=mT_sb[:], rhs=adg[:],
                                         start=True, stop=True)
                        z = wpool.tile([P, H], F32, tag="z")
                        nc.vector.tensor_tensor(
                            out=z[:], in0=gsl[:, HC:HC + H], in1=ad_ps[:],
                            op=mybir.AluOpType.add)
                        z2 = wpool.tile([P, H], F32, tag="z2")
                        nc.vector.tensor_scalar_mul(z2[:], z[:], 0.2)
                        lr = wpool.tile([P, H], F32, tag="lr")
                        nc.vector.tensor_tensor(out=lr[:], in0=z[:], in1=z2[:],
                                                op=mybir.AluOpType.max)
                        g_sb = wpool.tile([P, HC + H], F16, tag="g_sb")
                        nc.scalar.activation(
                            out=g_sb[:, HC:], in_=lr[:],
                            func=mybir.ActivationFunctionType.Exp,
                            bias=nshift[:])
                        nc.vector.tensor_tensor(
                            out=g_sb[:, :HC].rearrange("p (h c) -> p h c", c=CH),
                            in0=gsl[:, :HC].rearrange("p (h c) -> p h c", c=CH),
                            in1=g_sb[:, HC:][:, :, None].to_broadcast([P, H, CH]),
                            op=mybir.AluOpType.mult)
                        nc.tensor.matmul(u_ps[:], lhsT=m_sb[:], rhs=g_sb[:],
                                         start=(j == 0), stop=(j == ebt - 1))

                    s_sb = epool.tile([P, H], F32, tag="s_sb")
                    nc.vector.tensor_scalar_add(s_sb[:], u_ps[:, HC:], 1e-16)
                    r_sb = epool.tile([P, H], F32, tag="r_sb")
                    nc.vector.reciprocal(r_sb[:], s_sb[:])
                    zt = epool.tile([P, HC], F32, tag="zt")
                    nc.vector.tensor_tensor(
                        out=zt[:].rearrange("p (h c) -> p h c", c=CH),
                        in0=u_ps[:, :HC].rearrange("p (h c) -> p h c", c=CH),
                        in1=r_sb[:][:, :, None].to_broadcast([P, H, CH]),
                        op=mybir.AluOpType.mult)
                    zb = epool.tile([P, HC], F32, tag="zb")
                    nc.vector.tensor_tensor(out=zb[:], in0=zt[:], in1=b1bc[:],
                                            op=mybir.AluOpType.add)
                    t1 = epool.tile([P, HC], F32, tag="t1")
                    nc.vector.tensor_scalar(out=t1[:], in0=zb[:], scalar1=0.0,
                                            scalar2=None,
                                            op0=mybir.AluOpType.min)
                    t2 = epool.tile([P, HC], F32, tag="t2")
                    nc.scalar.activation(out=t2[:], in_=t1[:],
                                         func=mybir.ActivationFunctionType.Exp)
                    t3 = epool.tile([P, HC], F32, tag="t3")
                    nc.vector.tensor_scalar_add(t3[:], t2[:], -1.0)
                    h16 = epool.tile([P, HC], F16, tag="h16")
                    nc.vector.tensor_tensor(out=h16[:], in0=zb[:], in1=t3[:],
                                            op=mybir.AluOpType.max)
                    cols = min(P, npc - g * P)
                    for kk in range(2):
                        hT_ps = tpps.tile([P, P], F16, tag="T_ps")
                        nc.tensor.transpose(hT_ps[:],
                                            h16[:, kk * P:(kk + 1) * P],
                                            ident16[:])
                        hT_sb = epool.tile([P, P], F16, tag="hT_sb")
                        nc.vector.tensor_copy(hT_sb[:], hT_ps[:])
                        nc.sync.dma_start(
                            out=out_ap[kk * P:(kk + 1) * P,
                                       g * P:g * P + cols],
                            in_=hT_sb[:, :cols])
    nc.compile()
    return nc


# --------------------------------------------------------------------------
# launch B: layer 2
# --------------------------------------------------------------------------

def build_launch_b(cfg, eb_lo, eb_hi, num_devices=N_CORES):
    npad, ntiles = cfg["npad"], cfg["ntiles"]
    npc, ngroups = cfg["npc"], cfg["ngroups"]
    lo_tiles = cfg["lo_tiles"]
    lo_rows, hi_rows = cfg["lo_rows"], cfg["hi_rows"]
    lo_sent, hi_sent = cfg["lo_sent"], cfg["hi_sent"]
    ebt = eb_lo + eb_hi
    ncols = ngroups * ebt
    W2C = OUT + 2

    nc = bacc.Bacc("TRN2", target_bir_lowering=False, debug=False,
                   num_devices=num_devices)
    h_ap = nc.dram_tensor("hl1T", [HC, npad], F16, kind="ExternalInput").ap()
    w2_ap = nc.dram_tensor("w2ext", [HC, W2C], F16, kind="ExternalInput").ap()
    b2_ap = nc.dram_tensor("b2", [OUT], F32, kind="ExternalInput").ap()
    lo_ap = nc.dram_tensor("lo_idx", [P, ngroups * eb_lo * 8], I16,
                           kind="ExternalInput").ap()
    hi_ap = nc.dram_tensor("hi_idx", [P, ngroups * eb_hi * 8], I16,
                           kind="ExternalInput").ap()
    dstl_ap = nc.dram_tensor("dstl_cols", [P, ncols], F32, kind="ExternalInput").ap()
    ni_ap = nc.dram_tensor("nodeidx", [P, ngroups], I32, kind="ExternalInput").ap()
    y_ap = nc.dram_tensor("y", [npc, OUT], F32, kind="ExternalOutput").ap()

    hx2_lo = nc.dram_tensor("hx2_lo", [lo_rows, RW2], F16).ap()
    hx2_hi = nc.dram_tensor("hx2_hi", [hi_rows, RW2], F16).ap()
    ad2_g = nc.dram_tensor("ad2_glob", [npad, 1], F16).ap()

    TB = 4

    with tile.TileContext(nc) as tc:
        with tc.tile_pool(name="const", bufs=1) as cpool:
            ident32 = cpool.tile([P, P], F32)
            make_identity(nc, ident32[:])
            iota_f = cpool.tile([P, P], F32)
            nc.gpsimd.iota(iota_f[:], pattern=[[1, P]], base=0,
                           channel_multiplier=0,
                           allow_small_or_imprecise_dtypes=True)
            w2_sb = cpool.tile([P, 2, W2C], F16)
            nc.sync.dma_start(out=w2_sb[:, 0, :], in_=w2_ap[0:P, :])
            nc.sync.dma_start(out=w2_sb[:, 1, :], in_=w2_ap[P:2 * P, :])
            b2bc = cpool.tile([P, OUT], F32)
            nc.sync.dma_start(out=b2bc[:], in_=b2_ap[None, :].to_broadcast([P, OUT]))
            lo_sb = cpool.tile([P, ngroups * eb_lo * 8], I16)
            nc.sync.dma_start(out=lo_sb[:], in_=lo_ap[:])
            hi_sb = cpool.tile([P, ngroups * eb_hi * 8], I16)
            nc.sync.dma_start(out=hi_sb[:], in_=hi_ap[:])
            dstl = cpool.tile([P, ncols], F32)
            nc.sync.dma_start(out=dstl[:], in_=dstl_ap[:])
            nodei = cpool.tile([P, ngroups], I32)
            nc.sync.dma_start(out=nodei[:], in_=ni_ap[:])

            # ---------------- phase A: hx2 tables = hl1 @ W2ext ------------
            with (
                tc.tile_pool(name="pa_sbuf", bufs=3) as spool,
                tc.tile_pool(name="pa_out", bufs=3) as opool,
                tc.tile_pool(name="pa_psum", bufs=4, space="PSUM") as pps,
            ):
                for t0 in range(0, ntiles, TB):
                    tb = min(TB, ntiles - t0)
                    hT = spool.tile([P, 2, TB * P], F16, tag="hT")
                    for kk in range(2):
                        nc.sync.dma_start(
                            out=hT[:, kk, :tb * P],
                            in_=h_ap[kk * P:(kk + 1) * P,
                                     t0 * P:(t0 + tb) * P])
                    h24 = opool.tile([P, TB, RW2], F16, tag="h24")
                    nc.vector.memset(h24[:, :, W2C:], 0.0)
                    for ti in range(tb):
                        h2_ps = pps.tile([P, W2C], F32, tag="h2_ps")
                        for kk in range(2):
                            nc.tensor.matmul(
                                h2_ps[:],
                                lhsT=hT[:, kk, ti * P:(ti + 1) * P],
                                rhs=w2_sb[:, kk, :],
                                start=(kk == 0), stop=(kk == 1))
                        nc.scalar.copy(h24[:, ti, :W2C], h2_ps[:])
                    for ti in range(tb):
                        t = t0 + ti
                        if t < lo_tiles:
                            dst_tab, r0 = hx2_lo, t * P
                        else:
                            dst_tab, r0 = hx2_hi, (t - lo_tiles) * P
                        nc.sync.dma_start(out=dst_tab[r0:r0 + P, :],
                                          in_=h24[:, ti, :])
                        nc.sync.dma_start(out=ad2_g[t * P:(t + 1) * P, :],
                                          in_=h24[:, ti, OUT + 1:W2C])
                sent_row = opool.tile([P, RW2], F16, tag="sent_row")
                nc.vector.memset(sent_row[:], 0.0)
                nc.vector.memset(sent_row[:, OUT:OUT + 1], SENT_AS)
                nc.sync.dma_start(out=hx2_lo[lo_sent:lo_sent + P, :],
                                  in_=sent_row[:])
                nc.sync.dma_start(out=hx2_hi[hi_sent:hi_sent + 1, OUT:OUT + 1],
                                  in_=sent_row[0:1, OUT:OUT + 1])

            # ---------------- phase B: edge aggregation --------------------
            with (
                tc.tile_pool(name="pb_gather", bufs=2) as gpool,
                tc.tile_pool(name="pb_work", bufs=4) as wpool,
                tc.tile_pool(name="pb_ep", bufs=2) as epool,
                tc.tile_pool(name="pb_psum", bufs=2, space="PSUM") as upps,
                tc.tile_pool(name="pb_psmm", bufs=2, space="PSUM") as mpps,
                tc.tile_pool(name="pb_psumT", bufs=2, space="PSUM") as tpps,
            ):
                for g in range(ngroups):
                    glo = gpool.tile([P, eb_lo * RW2], F16, tag="glo")
                    nc.gpsimd.dma_gather(
                        out_ap=glo[:].rearrange("p (b w) -> p b w", w=RW2),
                        in_ap=hx2_lo[:],
                        idxs_ap=lo_sb[:, g * eb_lo * 8:(g + 1) * eb_lo * 8],
                        num_idxs=eb_lo * P, num_idxs_reg=eb_lo * P,
                        elem_size=RW2, single_packet=False)
                    ghi = gpool.tile([P, eb_hi * RW2], F16, tag="ghi")
                    nc.gpsimd.dma_gather(
                        out_ap=ghi[:].rearrange("p (b w) -> p b w", w=RW2),
                        in_ap=hx2_hi[:],
                        idxs_ap=hi_sb[:, g * eb_hi * 8:(g + 1) * eb_hi * 8],
                        num_idxs=eb_hi * P, num_idxs_reg=eb_hi * P,
                        elem_size=RW2, single_packet=False)
                    adg16 = gpool.tile([P, 1], F16, tag="adg16")
                    nc.gpsimd.indirect_dma_start(
                        out=adg16[:], out_offset=None, in_=ad2_g[:],
                        in_offset=bass.IndirectOffsetOnAxis(
                            ap=nodei[:, g:g + 1], axis=0))
                    adg = gpool.tile([P, 1], F32, tag="adg")
                    nc.vector.tensor_copy(adg[:], adg16[:])

                    u_ps = upps.tile([P, OUT + 1], F32, tag="u_ps")
                    for j in range(ebt):
                        c = g * ebt + j
                        if j < eb_lo:
                            gsl = glo[:, j * RW2:(j + 1) * RW2]
                        else:
                            jj = j - eb_lo
                            gsl = ghi[:, jj * RW2:(jj + 1) * RW2]
                        m_sb = wpool.tile([P, P], F32, tag="m_sb")
                        nc.vector.tensor_scalar(
                            out=m_sb[:], in0=iota_f[:],
                            scalar1=dstl[:, c:c + 1], scalar2=None,
                            op0=mybir.AluOpType.is_equal)
                        mT_ps = tpps.tile([P, P], F32, tag="T_ps")
                        nc.tensor.transpose(mT_ps[:], m_sb[:], ident32[:])
                        mT_sb = wpool.tile([P, P], F32, tag="mT_sb")
                        nc.vector.tensor_copy(mT_sb[:], mT_ps[:])
                        ad_ps = mpps.tile([P, 1], F32, tag="ad_ps")
                        nc.tensor.matmul(ad_ps[:], lhsT=mT_sb[:], rhs=adg[:],
                                         start=True, stop=True)
                        z = wpool.tile([P, 1], F32, tag="z")
                        nc.vector.tensor_tensor(
                            out=z[:], in0=gsl[:, OUT:OUT + 1], in1=ad_ps[:],
                            op=mybir.AluOpType.add)
                        z2 = wpool.tile([P, 1], F32, tag="z2")
                        nc.vector.tensor_scalar_mul(z2[:], z[:], 0.2)
                        lr = wpool.tile([P, 1], F32, tag="lr")
                        nc.vector.tensor_tensor(out=lr[:], in0=z[:], in1=z2[:],
                                                op=mybir.AluOpType.max)
                        g_sb = wpool.tile([P, OUT + 1], F32, tag="g_sb")
                        nc.scalar.activation(
                            out=g_sb[:, OUT:], in_=lr[:],
                            func=mybir.ActivationFunctionType.Exp, bias=0.0)
                        nc.vector.tensor_scalar(
                            out=g_sb[:, :OUT], in0=gsl[:, :OUT],
                            scalar1=g_sb[:, OUT:OUT + 1], scalar2=None,
                            op0=mybir.AluOpType.mult)
                        nc.tensor.matmul(u_ps[:], lhsT=m_sb[:], rhs=g_sb[:],
                                         start=(j == 0), stop=(j == ebt - 1))

                    s_sb = epool.tile([P, 1], F32, tag="s_sb")
                    nc.vector.tensor_scalar_add(s_sb[:], u_ps[:, OUT:], 1e-16)
                    r_sb = epool.tile([P, 1], F32, tag="r_sb")
                    nc.vector.reciprocal(r_sb[:], s_sb[:])
                    y_sb = epool.tile([P, OUT], F32, tag="y_sb")
                    nc.vector.tensor_scalar(
                        out=y_sb[:], in0=u_ps[:, :OUT],
                        scalar1=r_sb[:, 0:1], scalar2=None,
                        op0=mybir.AluOpType.mult)
                    yb = epool.tile([P, OUT], F32, tag="yb")
                    nc.vector.tensor_tensor(out=yb[:], in0=y_sb[:], in1=b2bc[:],
                                            op=mybir.AluOpType.add)
                    rows = min(P, npc - g * P)
                    nc.sync.dma_start(out=y_ap[g * P:g * P + rows, :],
                                      in_=yb[:rows, :])
    nc.compile()
    return nc


# --------------------------------------------------------------------------
# numpy emulation of the on-device algorithm (for logic validation)
# --------------------------------------------------------------------------

def emulate(x, W1ext, b1, W2ext, b2, cfg, tables, eb_lo, eb_hi):
    f16 = np.float16
    n_valid, npc, npad = cfg["n_valid"], cfg["npc"], cfg["npad"]
    ngroups, sent = cfg["ngroups"], cfg["sent"]
    n_cores, losplit = cfg["n_cores"], cfg["losplit"]
    ebt = eb_lo + eb_hi

    xpad = np.zeros((npad, D), np.float32)
    xpad[:n_valid] = x
    hx = (xpad.astype(f16).astype(np.float32) @
          W1ext.astype(np.float32)).astype(f16)     # [npad, 272]
    hxrow = np.zeros((npad + 1, RW1), f16)          # +1 = lo sentinel
    hxrow[:npad, :HC + H] = hx[:, :HC + H]
    hxrow[npad, HC:HC + H] = f16(SENT_AS)           # lo sentinel row
    hxrow[sent, HC:HC + H] = f16(SENT_AS)
    ad1 = hx[:, HC + H:].copy()
    ad1 = np.concatenate([ad1, np.zeros((1, H), f16)], axis=0)

    def row_of(tab, is_lo, idx):
        return np.where(is_lo, idx, np.minimum(idx + losplit, npad))

    hl1T = np.zeros((HC, npad), f16)
    for k in range(n_cores):
        tb = tables[k]
        lo_flat = tb["lo_idx"][:16].T.reshape(ngroups, eb_lo * P)
        hi_flat = tb["hi_idx"][:16].T.reshape(ngroups, eb_hi * P)
        for g in range(ngroups):
            U = np.zeros((P, HC + H), np.float32)
            adg = ad1[np.minimum(tb["nodeidx"][:, g], npad)].astype(np.float32)
            for j in range(ebt):
                if j < eb_lo:
                    idx = lo_flat[g, j * P:(j + 1) * P]
                    rows_idx = np.where(idx == cfg["lo_sent"], npad, idx)
                else:
                    jj = j - eb_lo
                    idx = hi_flat[g, jj * P:(jj + 1) * P]
                    rows_idx = idx + losplit
                rows = hxrow[rows_idx].astype(np.float32)
                dstl = tb["dstl_cols"][:, g * ebt + j]
                M = (np.arange(P)[None, :] == dstl[:, None]).astype(np.float32)
                ad_e = M @ adg
                z = rows[:, HC:HC + H] + ad_e
                lr = np.maximum(z, 0.2 * z)
                ex = np.exp(lr - SHIFT1).astype(f16).astype(np.float32)
                G = np.concatenate(
                    [(rows[:, :HC].reshape(P, H, CH) *
                      ex[:, :, None]).reshape(P, HC), ex], axis=1).astype(f16)
                U += M.T @ G.astype(np.float32)
            s = U[:, HC:] + 1e-16
            z = (U[:, :HC].reshape(P, H, CH) / s[:, :, None]).reshape(P, HC) + b1
            h1e = np.maximum(z, np.exp(np.minimum(z, 0)) - 1).astype(f16)
            cols = min(P, npc - g * P)
            hl1T[:, k * npc + g * P:k * npc + g * P + cols] = h1e.T[:, :cols]

    hx2 = (hl1T.T.astype(np.float32) @ W2ext.astype(np.float32)).astype(f16)
    hx2row = np.zeros((npad + 1, OUT + 2), f16)
    hx2row[:npad] = hx2
    hx2row[npad, OUT] = f16(SENT_AS)
    hx2row[sent, OUT] = f16(SENT_AS)
    ad2 = np.concatenate([hx2[:, OUT + 1:], np.zeros((1, 1), f16)], axis=0)

    y = np.zeros((n_valid, OUT), np.float32)
    for k in range(n_cores):
        tb = tables[k]
        lo_flat = tb["lo_idx"][:16].T.reshape(ngroups, eb_lo * P)
        hi_flat = tb["hi_idx"][:16].T.reshape(ngroups, eb_hi * P)
        for g in range(ngroups):
            U = np.zeros((P, OUT + 1), np.float32)
            adg = ad2[np.minimum(tb["nodeidx"][:, g], npad)].astype(np.float32)
            for j in range(ebt):
                if j < eb_lo:
                    idx = lo_flat[g, j * P:(j + 1) * P]
                    rows_idx = np.where(idx == cfg["lo_sent"], npad, idx)
                else:
                    jj = j - eb_lo
                    idx = hi_flat[g, jj * P:(jj + 1) * P]
                    rows_idx = idx + losplit
                rows = hx2row[rows_idx].astype(np.float32)
                dstl = tb["dstl_cols"][:, g * ebt + j]
                M = (np.arange(P)[None, :] == dstl[:, None]).astype(np.float32)
                ad_e = (M @ adg)[:, 0]
                z = rows[:, OUT] + ad_e
                lr = np.maximum(z, 0.2 * z)
                ex = np.exp(lr - SHIFT2)
                G = np.concatenate([rows[:, :OUT] * ex[:, None],
                                    ex[:, None]], axis=1)
                U += M.T @ G
            s = U[:, OUT] + 1e-16
            yg = U[:, :OUT] / s[:, None] + b2
            rows_n = min(P, npc - g * P)
            base = k * npc + g * P
            if base < n_valid:
                y[base:base + rows_n] = yg[:rows_n]
    return y


# --------------------------------------------------------------------------
# top-level kernel entry point (full inputs in, full output out)
# --------------------------------------------------------------------------

_CACHE = {}


def _run_with_retry(nc, in_maps, tries=3):
    """run_bass_kernel_spmd with retries; a transient device wedge usually
    clears on the next execution."""
    from concourse.bass_utils import run_bass_kernel_spmd
    last = None
    for attempt in range(tries):
        try:
            return run_bass_kernel_spmd(nc, in_maps,
                                        core_ids=list(range(len(in_maps))))
        except Exception as e:  # noqa: BLE001 - retry any runtime failure
            last = e
            import time as _time
            _time.sleep(2.0 * (attempt + 1))
    raise last


def kernel(**inputs):
    """Full-input GAT kernel on 8 Trainium2 NeuronCores.

    Takes the unsharded inputs of reference.setup_inputs(), distributes the
    work across 8 cores (dst-node graph partition), and returns the full
    [50000, 32] float32 output.
    """

    x = np.ascontiguousarray(np.asarray(inputs["x"], np.float32))
    ei = np.asarray(inputs["edge_index"])
    N = x.shape[0]
    npc = N // N_CORES
    assert npc * N_CORES == N

    src = np.concatenate([ei[0].astype(np.int64), np.arange(N, dtype=np.int64)])
    dst = np.concatenate([ei[1].astype(np.int64), np.arange(N, dtype=np.int64)])

    cfg = make_cfg(N, npc)
    tables, eb_lo, eb_hi = build_edge_tables(src, dst, cfg)
    W1ext, W2ext = fold_weights(
        inputs["W1"], inputs["a_src1"], inputs["a_dst1"],
        inputs["W2"], inputs["a_src2"], inputs["a_dst2"])
    b1 = np.asarray(inputs["b1"], np.float32)
    b2 = np.asarray(inputs["b2"], np.float32)

    key = (N, npc, eb_lo, eb_hi)
    if key not in _CACHE:
        _CACHE[key] = (build_launch_a(cfg, eb_lo, eb_hi),
                       build_launch_b(cfg, eb_lo, eb_hi))
    nc_a, nc_b = _CACHE[key]

    in_maps_a = [dict(
        x=x, w1ext=W1ext, b1=b1,
        lo_idx=tables[k]["lo_idx"], hi_idx=tables[k]["hi_idx"],
        dstl_cols=tables[k]["dstl_cols"], nodeidx=tables[k]["nodeidx"],
    ) for k in range(N_CORES)]
    res_a = _run_with_retry(nc_a, in_maps_a)

    hl1T_pad = np.zeros((HC, cfg["npad"]), np.float16)
    hl1T_pad[:, :N] = np.concatenate(
        [res_a.results[k]["hl1T"] for k in range(N_CORES)], axis=1)

    in_maps_b = [dict(
        hl1T=hl1T_pad, w2ext=W2ext, b2=b2,
        lo_idx=tables[k]["lo_idx"], hi_idx=tables[k]["hi_idx"],
        dstl_cols=tables[k]["dstl_cols"], nodeidx=tables[k]["nodeidx"],
    ) for k in range(N_CORES)]
    res_b = _run_with_retry(nc_b, in_maps_b)

    y = np.concatenate([res_b.results[k]["y"][:npc] for k in range(N_CORES)],
                       axis=0)
    return y.astype(np.float32)
